# revision 38
# baseline (speedup 1.0000x reference)
"""Block-diagonal 2x2 equalizer kernel for Trainium2 (8 NeuronCores), v4.

Per point (b, u, s, f) solves the 2x2 system M x = v by Cramer's rule:
    m_ij = h[b, pi[u], i, 0, 2u+j, s, f]   (only 1/4 of h is needed)
    det  = m00*m11 - m01*m10               (fp32: min |det| ~ 1.5e-4, so the
                                            det chain MUST stay fp32 - 16-bit
                                            h makes det cross zero)
    x0   = (m11*v0 - m01*v1) / det         (numerators tolerate fp16)
    x1   = (m00*v1 - m10*v0) / det

Sharding: pure data parallel over batch, 2 batches per core on 8 cores.
I/O per core: h planes fp32 (3.67MB), v planes fp16 (0.46MB), x out fp16
(0.46MB). Host does gather/transpose/dtype-pack only; all arithmetic is
on-device.

UNEVEN CHUNKS [224, 224, 448, 448, 448] over the 1792 free columns: the
small leading chunks give the pipeline a fast start (first A+B DMA is
only 3584B rows, fp32 chain starts ~4.4us) while the wide trailing
chunks amortize the ~75ns/op DVE instruction overhead once the input
stream is ahead of compute. (Uniform NCH=8 measured 15.7us DVE busy;
NCH=4 measured 14.7 busy but a 1.7us later start. This takes both.)

Engine split per chunk k (FC = FCs[k], WC = 2*FC):
  DVE:  QQ = {Af|Bf}*V_broadcast, one [128,2,WC] fp16 op at the HW 2x_1p
        rate (plain TensorTensor has a 2x uop program; fused
        TensorScalarPtr measured 1x); P = A*B wide fp32 {p0|p1};
        det = P0-P1. R = {q0|q2}-{q1|q3} and X = R*rdet_broadcast run
        joint over equal-size chunk pairs (0,1) and (2,3) as single
        [128,2,2,FC] strided ops (2x_1p holds for 4-dim APs); the last
        chunk runs per-chunk R/X so its store chain is short.
  ACT (parallel scalar engine): a [128,1] dummy activation first, forcing
        the one-time ACT_TABLE_LOAD (~1.3us) during the first DMA flight;
        then per chunk Af = cvt(A) wide, Bf = swapped-half cvts of B ->
        {m10f|m00f}, Reciprocal spline (fp32 in, fp16 out, 2.2e-5 rel
        err). Runs concurrently with DVE without contention (GPSIMD does
        contend - measured previously - and stays unused).
  SYNC: inputs on one ring, each chunk split into an A+B DMA (fp32) and
        a V DMA (fp16), issue order ab0, ab1, v0, ab2, v1, ... so the
        fp32 parts the det chain waits on are front-loaded and the DGE
        ring's one-time ~1.5us warm-up stall lands on deferrable
        traffic; then 2 pair stores + 1 last-chunk store, final outS
        wait.

Packing: A = {m11|m01}, B = {m00|m10}, V = {v0|v1} (fp16):
  P  = A*B = {m11*m00 | m01*m10} = {p0|p1}
  Af = {m11f|m01f},  Bf = {m10f|m00f}
  QQ = {Af|Bf}*{V|V} = {q0|q1|q3|q2}   (q0=m11f*v0 q1=m01f*v1
                                        q2=m00f*v1 q3=m10f*v0)
  R  = {q0|q2} - {q1|q3} = {r0|r1} per chunk, X = R*rdet = {x0|x1}

Dataflow rules: every SBUF region written exactly once (no WAR hazards),
all waits standalone wait_ge (walrus single-wait rule), cross-engine waits
get >= 1 chunk of slack so semaphore propagation never stalls a hot pipe.
Block(no_gpsimd_drain=True) skips the slow gpsimd dge_drain at teardown.
"""

from contextlib import ExitStack

import numpy as np

import concourse.bass as bass
import concourse.mybir as mybir
from concourse.bass_utils import run_bass_kernel_spmd

# Problem shapes (hardcoded per contract)
B, U, A, NTX, T, S, F = 16, 4, 2, 1, 8, 14, 2048
SF = S * F               # 28672
NCORES = 8
BPC = B // NCORES        # 2 batches per core
PTS = BPC * U * SF       # 229376 points per core
COLS = PTS // 128        # 1792

FCs = [224, 224, 224, 224, 448, 448]     # uneven chunk widths, sum = COLS
NCH = len(FCs)
OFF = [sum(FCs[:k]) for k in range(NCH)]  # column offset per chunk
JOINT = [(0, 1), (2, 3)]                  # equal-size pairs with joint R/X
PERCHUNK = [4, 5]                         # trailing chunks with own R/X
ROWB = 20 * COLS                          # input bytes per partition row
ABOFF = [16 * o for o in OFF]             # A+B fp32 segment byte offsets
VBASE = 16 * COLS
VOFF = [VBASE + 4 * o for o in OFF]       # V fp16 segment byte offsets

TRACE = False
LAST_RESULTS = None

f32 = mybir.dt.float32
f16 = mybir.dt.float16
u8 = mybir.dt.uint8


def _build_nc():
    nc = bass.Bass("TRN2")
    dIn = nc.dram_tensor("dIn", [128, ROWB], u8, kind="ExternalInput")
    xO = nc.dram_tensor("xO", [128, 2 * COLS], f16, kind="ExternalOutput")

    with ExitStack() as ctx:
        sb = lambda name, w, dt: ctx.enter_context(nc.sbuf_tensor(name, [128, w], dt))
        tIn = sb("tIn", ROWB, u8)
        tABf = sb("tABf", 4 * COLS, f16)
        tP = sb("tP", 2 * COLS, f32)
        tDet = sb("tDet", COLS, f32)
        tRda = sb("tRda", COLS, f16)
        tWarm = sb("tWarm", 1, f16)
        tQa = sb("tQa", 4 * COLS, f16)
        tRa = sb("tRa", 2 * COLS, f16)
        tX = sb("tX", 2 * COLS, f16)

        def seg(t, o, w, dt=None):
            s = t[:, o:o + w]
            return s.bitcast(dt) if dt else s

        vA = [seg(tIn, ABOFF[k], 8 * FCs[k], f32) for k in range(NCH)]
        vB = [seg(tIn, ABOFF[k] + 8 * FCs[k], 8 * FCs[k], f32) for k in range(NCH)]
        vV = [seg(tIn, VOFF[k], 4 * FCs[k], f16) for k in range(NCH)]

        abS = [ctx.enter_context(nc.semaphore(f"abS{k}")) for k in range(NCH)]
        vS = [ctx.enter_context(nc.semaphore(f"vS{k}")) for k in range(NCH)]
        dveS = ctx.enter_context(nc.semaphore("dveS"))
        actS = ctx.enter_context(nc.semaphore("actS"))
        outS = ctx.enter_context(nc.semaphore("outS"))

        # --- semaphore threshold bookkeeping (must mirror program order) ---
        det_idx = [0] * NCH
        cvt_idx = [0] * NCH
        recip_idx = [0] * NCH
        x_idx = [0] * 4          # pair0, pair1, chunk4, chunk5
        dc = 0
        for t in range(NCH + 1):
            if 1 <= t <= NCH:
                dc += 1                       # QQ(t-1)
            if t < NCH:
                dc += 2                       # P(t), det(t)
                det_idx[t] = dc
            if t == 2 or t == 4:
                dc += 2                       # joint R, X
                x_idx[t // 2 - 1] = dc
            if t - 1 in PERCHUNK:
                dc += 2                       # per-chunk R, X
                x_idx[2 + PERCHUNK.index(t - 1)] = dc
        ac = 0
        for t in range(NCH + 1):
            if 1 <= t <= NCH:
                ac += 1                       # recip(t-1)
                recip_idx[t - 1] = ac
            if t < NCH:
                ac += 3                       # Af, Bf halves
                cvt_idx[t] = ac

        with nc.Block(no_gpsimd_drain=True) as block:

            @block.scalar
            def _(scalar):
                # dummy activation with no data dependency: forces the one-time
                # ACT_TABLE_LOAD to run during the first DMA flight
                scalar.copy(tWarm[:], nc.const_aps.aps[(f32, 0.0)])
                for t in range(NCH + 1):
                    if 1 <= t <= NCH:
                        k = t - 1
                        fc = FCs[k]
                        scalar.wait_ge(dveS, det_idx[k])
                        scalar.add_instruction(
                            mybir.InstActivation(
                                name=nc.get_next_instruction_name(),
                                func=mybir.ActivationFunctionType.Reciprocal,
                                ins=[
                                    scalar.lower_ap(tDet[:, OFF[k]:OFF[k] + fc]),
                                    mybir.ImmediateValue(dtype=f32, value=0.0),
                                    mybir.ImmediateValue(dtype=f32, value=1.0),
                                    mybir.ImmediateValue(dtype=f32, value=0.0),
                                ],
                                outs=[
                                    scalar.lower_ap(tRda[:, OFF[k]:OFF[k] + fc])
                                ],
                            )
                        ).then_inc(actS, 1)
                    if t < NCH:
                        k = t
                        fc = FCs[k]
                        o4 = 4 * OFF[k]
                        scalar.wait_ge(abS[k], 16)
                        scalar.copy(tABf[:, o4:o4 + 2 * fc], vA[k]).then_inc(actS, 1)
                        scalar.copy(
                            tABf[:, o4 + 2 * fc:o4 + 3 * fc], vB[k][:, fc:]
                        ).then_inc(actS, 1)
                        scalar.copy(
                            tABf[:, o4 + 3 * fc:o4 + 4 * fc], vB[k][:, :fc]
                        ).then_inc(actS, 1)

            @block.sync
            def _(sync):
                def _ab(k):
                    sync.dma_start(
                        out=tIn[:, ABOFF[k]:ABOFF[k] + 16 * FCs[k]],
                        in_=dIn[:, ABOFF[k]:ABOFF[k] + 16 * FCs[k]],
                    ).then_inc(abS[k], 16)

                def _v(k):
                    sync.dma_start(
                        out=tIn[:, VOFF[k]:VOFF[k] + 4 * FCs[k]],
                        in_=dIn[:, VOFF[k]:VOFF[k] + 4 * FCs[k]],
                    ).then_inc(vS[k], 16)

                _ab(0)
                _ab(1)
                _v(0)
                for k in range(2, NCH):
                    _ab(k)
                    _v(k - 1)
                _v(NCH - 1)

                # stores: pair (0,1), pair (2,3), chunk4, chunk5
                bounds = [
                    (2 * OFF[0], 2 * (OFF[1] + FCs[1])),
                    (2 * OFF[2], 2 * (OFF[3] + FCs[3])),
                ] + [(2 * OFF[k], 2 * (OFF[k] + FCs[k])) for k in PERCHUNK]
                for i, (a, b) in enumerate(bounds):
                    sync.wait_ge(dveS, x_idx[i])
                    sync.dma_start(out=xO[:, a:b], in_=tX[:, a:b]).then_inc(outS, 16)
                sync.wait_ge(outS, len(bounds) * 16)

            @block.vector
            def _(vector):
                for t in range(NCH + 1):
                    if 1 <= t <= NCH:
                        k = t - 1
                        fc = FCs[k]
                        o4 = 4 * OFF[k]
                        vector.wait_ge(actS, cvt_idx[k])
                        vector.wait_ge(vS[k], 16)
                        qq = tQa[:, o4:o4 + 4 * fc].rearrange(
                            "p (a c) -> p a c", a=2, c=2 * fc
                        )
                        vbc = vV[k].unsqueeze(1).broadcast_to((128, 2, 2 * fc))
                        abf = tABf[:, o4:o4 + 4 * fc].rearrange(
                            "p (a c) -> p a c", a=2, c=2 * fc
                        )
                        vector.tensor_mul(qq, abf, vbc).then_inc(dveS, 1)
                    if t < NCH:
                        k = t
                        fc = FCs[k]
                        o2 = 2 * OFF[k]
                        vector.wait_ge(abS[k], 16)
                        vector.tensor_mul(
                            tP[:, o2:o2 + 2 * fc], vA[k], vB[k]
                        ).then_inc(dveS, 1)
                        vector.tensor_sub(
                            tDet[:, OFF[k]:OFF[k] + fc],
                            tP[:, o2:o2 + fc],
                            tP[:, o2 + fc:o2 + 2 * fc],
                        ).then_inc(dveS, 1)
                    if t == 2 or t == 4:
                        ka, kb = JOINT[t // 2 - 1]
                        fc = FCs[ka]
                        o4 = 4 * OFF[ka]
                        vector.wait_ge(actS, recip_idx[kb])
                        q8 = tQa[:, o4:o4 + 8 * fc].rearrange(
                            "p (a b c) -> p a b c", a=2, b=4, c=fc
                        )
                        rr = tRa[:, 2 * OFF[ka]:2 * OFF[ka] + 4 * fc].rearrange(
                            "p (a b c) -> p a b c", a=2, b=2, c=fc
                        )
                        vector.tensor_sub(
                            rr, q8[:, :, 0::3], q8[:, :, 1:3]
                        ).then_inc(dveS, 1)
                        xx = tX[:, 2 * OFF[ka]:2 * OFF[ka] + 4 * fc].rearrange(
                            "p (a b c) -> p a b c", a=2, b=2, c=fc
                        )
                        rdb = (
                            tRda[:, OFF[ka]:OFF[ka] + 2 * fc]
                            .rearrange("p (a c) -> p a c", a=2, c=fc)
                            .unsqueeze(2)
                            .broadcast_to((128, 2, 2, fc))
                        )
                        vector.tensor_mul(xx, rr, rdb).then_inc(dveS, 1)
                    if t - 1 in PERCHUNK:
                        k = t - 1
                        fc = FCs[k]
                        o4 = 4 * OFF[k]
                        vector.wait_ge(actS, recip_idx[k])
                        q4 = tQa[:, o4:o4 + 4 * fc].rearrange(
                            "p (a c) -> p a c", a=4, c=fc
                        )
                        rr = tRa[:, 2 * OFF[k]:2 * OFF[k] + 2 * fc].rearrange(
                            "p (a c) -> p a c", a=2, c=fc
                        )
                        vector.tensor_sub(rr, q4[:, 0::3], q4[:, 1:3]).then_inc(
                            dveS, 1
                        )
                        xx = tX[:, 2 * OFF[k]:2 * OFF[k] + 2 * fc].rearrange(
                            "p (a c) -> p a c", a=2, c=fc
                        )
                        rdb = (
                            tRda[:, OFF[k]:OFF[k] + fc]
                            .unsqueeze(1)
                            .broadcast_to((128, 2, fc))
                        )
                        vector.tensor_mul(xx, rr, rdb).then_inc(dveS, 1)

    return nc


def make_in_maps(y, h, precoding_ind):
    """Host-side gather + byte-pack. Returns per-core input maps."""
    y = np.asarray(y)
    h = np.asarray(h)
    pi = np.asarray(precoding_ind).astype(np.int64)

    hg = h[:, pi[0]]                                     # [B, U, A, NTX, T, S, F]
    hsel = np.stack(
        [hg[:, u, :, 0, 2 * u:2 * u + 2] for u in range(U)], axis=1
    )                                                    # [B, U, A(i), 2(j), S, F]
    hsel = np.ascontiguousarray(hsel).astype(np.float32)
    yr = np.ascontiguousarray(y).astype(np.float32)      # [B, U, A, S, F]

    in_maps = []
    for c in range(NCORES):
        b0 = c * BPC
        hs = hsel[b0:b0 + BPC]
        ys = yr[b0:b0 + BPC]
        m00 = np.ascontiguousarray(hs[:, :, 0, 0]).reshape(128, COLS)
        m01 = np.ascontiguousarray(hs[:, :, 0, 1]).reshape(128, COLS)
        m10 = np.ascontiguousarray(hs[:, :, 1, 0]).reshape(128, COLS)
        m11 = np.ascontiguousarray(hs[:, :, 1, 1]).reshape(128, COLS)
        v0 = np.ascontiguousarray(ys[:, :, 0]).reshape(128, COLS).astype(np.float16)
        v1 = np.ascontiguousarray(ys[:, :, 1]).reshape(128, COLS).astype(np.float16)
        parts = []
        for k in range(NCH):
            s = slice(OFF[k], OFF[k] + FCs[k])
            ab = np.concatenate(
                [m11[:, s], m01[:, s], m00[:, s], m10[:, s]], axis=1
            )                                            # A then B, [128, 4*FC] f32
            parts.append(ab.view(np.uint8))
        for k in range(NCH):
            s = slice(OFF[k], OFF[k] + FCs[k])
            v = np.concatenate([v0[:, s], v1[:, s]], axis=1)  # [128, 2*FC] f16
            parts.append(v.view(np.uint8))
        dIn = np.concatenate(parts, axis=1)
        assert dIn.shape == (128, ROWB)
        in_maps.append({"dIn": np.ascontiguousarray(dIn)})
    return in_maps


def assemble_output(results):
    """Per-core xO [128, 2*COLS] f16 -> full [B, U, A, S, F] f32."""
    out = np.empty((B, U, A, S, F), np.float32)
    for c in range(NCORES):
        xo = np.asarray(results[c]["xO"]).astype(np.float32)
        x0 = np.empty((128, COLS), np.float32)
        x1 = np.empty((128, COLS), np.float32)
        for k in range(NCH):
            o2, fc = 2 * OFF[k], FCs[k]
            x0[:, OFF[k]:OFF[k] + fc] = xo[:, o2:o2 + fc]
            x1[:, OFF[k]:OFF[k] + fc] = xo[:, o2 + fc:o2 + 2 * fc]
        out[c * BPC:(c + 1) * BPC, :, 0] = x0.reshape(BPC, U, S, F)
        out[c * BPC:(c + 1) * BPC, :, 1] = x1.reshape(BPC, U, S, F)
    return out


def kernel(y, h, precoding_ind):
    global LAST_RESULTS
    in_maps = make_in_maps(y, h, precoding_ind)
    nc = _build_nc()
    res = run_bass_kernel_spmd(nc, in_maps, list(range(NCORES)), trace=TRACE)
    LAST_RESULTS = res
    return assemble_output(res.results)
